# revision 53
# baseline (speedup 1.0000x reference)
"""Trainium2 Bass kernel for nn_EGMFA_12249246728581.

Mathematical reduction used here:
  The reference computes asm = irfft2(rfft2(pred_proc) * rfft2(feat_proc))
  summed over all spatial positions. The spatial sum of a circular
  convolution equals the product of the operands' spatial sums (DC Fourier
  coefficient), so asm[b,n,c] = S_pred[b,n] * S_feat[b,c].

  The spatial sum of a zero-padded depthwise conv collapses via
  inclusion-exclusion to a weighted combination of the input's total sum,
  edge-row/col strip sums and corner pixels.  So the only O(H*W) work is:
    - per-channel row sums + edge-column strip sums of `feat`  (device)
    - bilinear x2 upsample + sigmoid chain of `pred`, reduced the same way
      (device)
  Everything else is O(C)-sized and is combined on host in float64.

Device program (SPMD, one batch element per core; ~47us of HBM streaming
is the roofline and everything else overlaps it):
  feat[b] viewed as [128, 32768] (partition p = channel p//2, row-half p%2)
    - streamed in 10 tail-shrinking chunks on the SP hwdge ring;
      per chunk one 2x-mode tensor_scalar accum gives the chunk total per
      partition, and strided reduces accumulate the first/last-5-column
      strips (needed for the horizontal dwconv window sums).
  pred[b] [128,128] (emitted first so it hides under the feat stream):
    - vertical bilinear x2 via two PE matmuls with interpolation matrices,
      horizontal via shifted scalar_tensor_tensor ops, sigmoid chain on
      ACT, all reductions fused into accum_out outputs.
  Edge-row sums, corner pixels and the last 6 local rows' strip
  contributions come from tiny host-side slices of the raw input.
"""

import numpy as np

BS, C, H, W = 8, 64, 256, 256
GC, FC = 8, 64
CID = C - 3 * GC  # 40
LN_EPS = 1e-5
N_CORES = 8
FREE = (C * H * W) // 128       # 32768 floats per partition

# local rows (of 256 floats) per chunk; every chunk reduces to one
# 2x-mode total per partition.  Edge-row sums (channel rows 0-4/251-255)
# come from the host (0.04% of the data).  Tail-shrinking sizes keep the
# post-stream device work to one tiny tensor_scalar + one short DMA.
CHUNK_ROWS = [18, 18, 18, 18, 18, 18, 8, 6, 4, 2]
N_CHUNKS = len(CHUNK_ROWS)
N_TAIL = 2                      # last chunks: totals only, strips on host
TAIL_ROW0 = 128 - sum(CHUNK_ROWS[-N_TAIL:])   # 122
# stats columns: [0:9] chunk 0-8 totals, [9:19] cstrip accum (chunks 0-7,
# i.e. channel rows 0-121 and 128-249), [19] chunk-9 total.  The strip
# contribution of the tail chunks' rows (122-127, 250-255) is added on
# host from the raw input.
CSTRIP0 = N_CHUNKS - 1          # 9
STAT_COLS = CSTRIP0 + 11        # 20

_nc_cache = None


def _upsample_mats():
    """U_e/U_o: [128,128] f32, out_even = U_e.T @ x, out_odd = U_o.T @ x.
    Half-pixel-center bilinear x2 (align_corners=False)."""
    ue = np.zeros((128, 128), np.float32)
    uo = np.zeros((128, 128), np.float32)
    for i in range(128):
        if i == 0:
            ue[0, 0] = 1.0
        else:
            ue[i - 1, i] = 0.25
            ue[i, i] = 0.75
        if i == 127:
            uo[127, 127] = 1.0
        else:
            uo[i, i] = 0.75
            uo[i + 1, i] = 0.25
    return ue, uo


def _build_program(reps=1):
    import concourse.tile as tile
    from concourse import bacc, mybir

    f32 = mybir.dt.float32
    AX = mybir.AxisListType.X
    ALU = mybir.AluOpType
    ACT_SIG = mybir.ActivationFunctionType.Sigmoid

    nc = bacc.Bacc(None, target_bir_lowering=False)
    feat_in = nc.dram_tensor("feat", [128, FREE], f32, kind="ExternalInput")
    # paux = [pred | U_even | U_odd] concatenated so one DMA loads all three
    paux_in = nc.dram_tensor("paux", [128, 384], f32, kind="ExternalInput")
    # fstat = [rsum cols | col strips] in one tile -> one tail DMA
    fstat_out = nc.dram_tensor("fstat", [128, STAT_COLS], f32, kind="ExternalOutput")
    psmall_out = nc.dram_tensor("psmall", [128, 16], f32, kind="ExternalOutput")

    with tile.TileContext(nc) as tc:
        with (
            tc.tile_pool(name="feat", bufs=3) as feat_pool,
            tc.tile_pool(name="feat_sm", bufs=5) as feat_sm_pool,
            tc.tile_pool(name="acc", bufs=1) as acc_pool,
            tc.tile_pool(name="tmp", bufs=2) as tmp_pool,
            tc.tile_pool(name="pred", bufs=1) as pred_pool,
            tc.tile_pool(name="psum", bufs=1, space="PSUM") as psum_pool,
        ):
          assert sum(CHUNK_ROWS) == 128

          for _rep in range(reps):
            # ---------------- pred pipeline (emitted first so all its
            # compute lands in engine idle time under the feat stream;
            # its DVE ops retire before chunk 0's DMA even completes) ----
            psmall = acc_pool.tile([128, 16], f32)
            nc.vector.memset(psmall[:], 0.0)

            # tiny load on the ACT hwdge ring, so the SP ring starts the
            # feat stream at t=0 with nothing in front of it.
            paux_t = pred_pool.tile([128, 384], f32)
            nc.scalar.dma_start(paux_t[:], paux_in[:, :])
            pred_t = paux_t[:, 0:128]
            ue_t = paux_t[:, 128:256]
            uo_t = paux_t[:, 256:384]

            a_e_p = psum_pool.tile([128, 128], f32)
            a_o_p = psum_pool.tile([128, 128], f32)
            nc.tensor.matmul(a_e_p[:], ue_t, pred_t, start=True, stop=True)
            nc.tensor.matmul(a_o_p[:], uo_t, pred_t, start=True, stop=True)
            a_e = pred_pool.tile([128, 128], f32)
            a_o = pred_pool.tile([128, 128], f32)
            nc.vector.tensor_copy(a_e[:], a_e_p[:])
            nc.vector.tensor_copy(a_o[:], a_o_p[:])

            # phases: (row parity, col parity); P = pred_up * 4/3, then
            # sigmoid(P * 0.75) = pred_up sigmoid.
            phase_src = {"ee": (a_e, "e"), "eo": (a_e, "o"),
                         "oe": (a_o, "e"), "oo": (a_o, "o")}
            phase_idx = {"ee": 0, "eo": 1, "oe": 2, "oo": 3}
            pa_tiles = {}
            for ph, (a, cpar) in phase_src.items():
                p_t = pred_pool.tile([128, 128], f32, tag=f"p_{ph}")
                if cpar == "e":
                    # out col j: j==0 -> A[:,0]*(4/3); j>0 -> A[:,j-1]/3 + A[:,j]
                    nc.vector.scalar_tensor_tensor(
                        p_t[:, 1:128], a[:, 0:127], 1.0 / 3.0, a[:, 1:128],
                        op0=ALU.mult, op1=ALU.add)
                    nc.vector.scalar_tensor_tensor(
                        p_t[:, 0:1], a[:, 0:1], 1.0 / 3.0, a[:, 0:1],
                        op0=ALU.mult, op1=ALU.add)
                else:
                    nc.vector.scalar_tensor_tensor(
                        p_t[:, 0:127], a[:, 1:128], 1.0 / 3.0, a[:, 0:127],
                        op0=ALU.mult, op1=ALU.add)
                    nc.vector.scalar_tensor_tensor(
                        p_t[:, 127:128], a[:, 127:128], 1.0 / 3.0, a[:, 127:128],
                        op0=ALU.mult, op1=ALU.add)
                i = phase_idx[ph]
                s_t = pred_pool.tile([128, 128], f32, tag=f"s_{ph}")
                # pred_s = sigmoid(0.75*P); accum -> total per partition
                nc.scalar.activation(s_t[:], p_t[:], ACT_SIG, scale=0.75,
                                     accum_out=psmall[:, 4 + i:5 + i])
                g_t = pred_pool.tile([128, 128], f32, tag=f"g_{ph}")
                nc.scalar.activation(g_t[:], s_t[:], ACT_SIG)  # sigmoid(pred_s)
                t2 = pred_pool.tile([128, 128], f32, tag=f"t2_{ph}")
                nc.vector.tensor_mul(t2[:], s_t[:], g_t[:])    # pred_s * sig2
                pa = pred_pool.tile([128, 128], f32, tag=f"pa_{ph}")
                # pa = 2*pred_s - pred_s*sig2; accum -> pa row sums
                nc.vector.scalar_tensor_tensor(
                    pa[:], s_t[:], 2.0, t2[:],
                    op0=ALU.mult, op1=ALU.subtract,
                    accum_out=psmall[:, i:i + 1])
                pa_tiles[ph] = pa

            # edge columns of pa (col 0 from even-col phases, col 255 from odd)
            nc.vector.tensor_copy(psmall[:, 8:9], pa_tiles["ee"][:, 0:1])
            nc.vector.tensor_copy(psmall[:, 9:10], pa_tiles["oe"][:, 0:1])
            nc.vector.tensor_copy(psmall[:, 10:11], pa_tiles["eo"][:, 127:128])
            nc.vector.tensor_copy(psmall[:, 11:12], pa_tiles["oo"][:, 127:128])
            # psmall is complete well before the feat stream ends;
            # Pool (SWDGE) ring keeps it off the busy SP/ACT rings.
            nc.gpsimd.dma_start(psmall_out[:, :], psmall[:])
            del pa_tiles

            # ---------------- feat reduction ----------------
            stats = acc_pool.tile([128, STAT_COLS], f32)
            cstrip = stats[:, CSTRIP0:CSTRIP0 + 10]

            row0 = 0
            for k, nrows in enumerate(CHUNK_ROWS):
                fw = nrows * 256
                last = (k == N_CHUNKS - 1)
                if nrows >= 16:
                    ch = feat_pool.tile([128, fw], f32, tag="ch_big")
                else:
                    ch = feat_sm_pool.tile([128, fw], f32, tag="ch_small")
                nc.sync.dma_start(ch[:], feat_in[:, row0 * 256:(row0 + nrows) * 256])
                # one scalar per partition, 2x-mode tensor_scalar
                tot_col = STAT_COLS - 1 if last else k
                scratch = tmp_pool.tile([128, 18 * 256], f32, tag="scratch")
                nc.vector.tensor_scalar(
                    scratch[:, 0:fw], ch[:], 1.0, 0.0,
                    op0=ALU.mult, op1=ALU.add,
                    accum_out=stats[:, tot_col:tot_col + 1])
                # column strips: [128, cols, rows(stride 256)] reduce innermost
                ch_c = ch[:].rearrange("p (r w) -> p w r", w=256)
                if k >= N_CHUNKS - N_TAIL:
                    pass                       # tail strips come from host
                elif k == 0:
                    nc.vector.reduce_sum(cstrip[:, 0:5], ch_c[:, 0:5, :], axis=AX)
                    nc.vector.reduce_sum(cstrip[:, 5:10], ch_c[:, 251:256, :], axis=AX)
                else:
                    tl = tmp_pool.tile([128, 5], f32)
                    nc.vector.reduce_sum(tl[:], ch_c[:, 0:5, :], axis=AX)
                    nc.vector.tensor_add(cstrip[:, 0:5], cstrip[:, 0:5], tl[:])
                    th = tmp_pool.tile([128, 5], f32)
                    nc.vector.reduce_sum(th[:], ch_c[:, 251:256, :], axis=AX)
                    nc.vector.tensor_add(cstrip[:, 5:10], cstrip[:, 5:10], th[:])
                # stream completed totals out on the ACT hwdge ring
                if k < 6:
                    nc.scalar.dma_start(fstat_out[:, k:k + 1],
                                        stats[:, k:k + 1])
                row0 += nrows

            # bulk tail (ready once chunk 8 retires), then the short last
            # column that only depends on the final 2-row chunk's total.
            nc.scalar.dma_start(fstat_out[:, 6:STAT_COLS - 1],
                                stats[:, 6:STAT_COLS - 1])
            nc.scalar.dma_start(fstat_out[:, STAT_COLS - 1:STAT_COLS],
                                stats[:, STAT_COLS - 1:STAT_COLS])

    nc.compile()
    return nc


def _get_program():
    global _nc_cache
    if _nc_cache is None:
        _nc_cache = _build_program()
    return _nc_cache


def _sigmoid(x):
    return 1.0 / (1.0 + np.exp(-x))





def kernel(feat, head, pred, params):
    from concourse.bass_utils import run_bass_kernel_spmd

    feat = np.ascontiguousarray(feat, np.float32)
    head = np.asarray(head)
    pred = np.ascontiguousarray(pred, np.float32)

    ue, uo = _upsample_mats()
    in_maps = []
    for b in range(N_CORES):
        in_maps.append({
            "feat": feat[b].reshape(128, FREE),
            "paux": np.concatenate([pred[b, 0], ue, uo], axis=1),
        })

    nc = _get_program()
    results = run_bass_kernel_spmd(nc, in_maps, list(range(N_CORES))).results

    fstat = np.stack([r["fstat"] for r in results]).astype(np.float64)
    psmall = np.stack([r["psmall"] for r in results]).astype(np.float64)
    # totals: chunk 0-8 at cols [0:9], last chunk at the final col
    Th = fstat[:, :, 0:CSTRIP0].sum(-1) + fstat[:, :, STAT_COLS - 1]
    # device strips cover channel rows 0-121 / 128-249; add the tail rows
    # (122-127, 250-255) from the raw input.
    cstrip = fstat[:, :, CSTRIP0:CSTRIP0 + 10]

    p = {k: np.asarray(v, np.float64) for k, v in params.items()}

    # ---- reassemble feat sums ----
    # Th = total per half-channel; partition 2c = channel c rows 0-127,
    # partition 2c+1 = rows 128-255.
    T = Th[:, 0::2] + Th[:, 1::2]                  # [B, C]
    # edge-row sums straight from the raw input (tiny slices)
    rows_lo = feat[:, :, 0:5, :].astype(np.float64).sum(-1)    # rows 0..4
    rows_hi = feat[:, :, 251:256, :].astype(np.float64).sum(-1)  # 251..255
    cs = cstrip[:, 0::2, :] + cstrip[:, 1::2, :]   # [B, C, 10] cols 0-4,251-255
    for r0 in (TAIL_ROW0, 128 + TAIL_ROW0):        # rows 122-127, 250-255
        slab = feat[:, :, r0:r0 + (128 - TAIL_ROW0), :].astype(np.float64)
        cs[..., 0:5] += slab[..., 0:5].sum(2)
        cs[..., 5:10] += slab[..., 251:256].sum(2)

    S_feat = T.copy()

    # y_hw: channels 40-47, 3x3 pad 1
    for ci in range(GC):
        c = CID + ci
        w = p['inc_hw_w'][ci, 0]
        s = np.zeros(BS)
        for ki in range(3):
            for kj in range(3):
                dr, dc = ki - 1, kj - 1
                win = T[:, c].copy()
                if dr == 1:
                    win -= rows_lo[:, c, 0]
                elif dr == -1:
                    win -= rows_hi[:, c, 4]
                if dc == 1:
                    win -= cs[:, c, 0]
                elif dc == -1:
                    win -= cs[:, c, 9]
                if dr != 0 and dc != 0:
                    rr = 0 if dr == 1 else 255
                    cc = 0 if dc == 1 else 255
                    win += feat[:, c, rr, cc].astype(np.float64)
                s += w[ki, kj] * win
        S_feat[:, c] = s + H * W * p['inc_hw_b'][ci]

    # y_w: channels 48-55, 1x11 pad (0,5)
    for ci in range(GC):
        c = CID + GC + ci
        w = p['inc_w_w'][ci, 0, 0]                 # [11]
        s = np.zeros(BS)
        for kj in range(11):
            d = kj - 5
            win = T[:, c].copy()
            if d > 0:
                win -= cs[:, c, :d].sum(-1)        # cols 0..d-1
            elif d < 0:
                win -= cs[:, c, 10 + d:].sum(-1)   # cols 256+d..255
            s += w[kj] * win
        S_feat[:, c] = s + H * W * p['inc_w_b'][ci]

    # y_h: channels 56-63, 11x1 pad (5,0)
    for ci in range(GC):
        c = CID + 2 * GC + ci
        w = p['inc_h_w'][ci, 0, :, 0]              # [11]
        s = np.zeros(BS)
        for ki in range(11):
            d = ki - 5
            win = T[:, c].copy()
            if d > 0:
                win -= rows_lo[:, c, :d].sum(-1)
            elif d < 0:
                win -= rows_hi[:, c, 5 + d:].sum(-1)
            s += w[ki] * win
        S_feat[:, c] = s + H * W * p['inc_h_b'][ci]

    # ---- pred scalar ----
    T_ps = psmall[:, :, 4:8].sum((1, 2))           # total of sigmoid(up)
    T_pa = psmall[:, :, 0:4].sum((1, 2))
    row0 = psmall[:, 0, 0] + psmall[:, 0, 1]
    row255 = psmall[:, 127, 2] + psmall[:, 127, 3]
    col0 = psmall[:, :, 8:10].sum((1, 2))
    col255 = psmall[:, :, 10:12].sum((1, 2))
    pv = pred[:, 0][:, [0, 0, 127, 127], [0, 127, 0, 127]].astype(np.float64)
    sc = _sigmoid(pv)
    pcorn = sc * (1.0 - _sigmoid(sc)) + sc         # pa at the 4 corners

    dw = p['dw_w'][0, 0]                           # [3,3]
    S_pred = T_ps + H * W * p['dw_b'][0]
    corner_map = {(1, 1): 0, (1, -1): 1, (-1, 1): 2, (-1, -1): 3}
    for ki in range(3):
        for kj in range(3):
            dr, dc = ki - 1, kj - 1
            win = T_pa.copy()
            if dr == 1:
                win -= row0
            elif dr == -1:
                win -= row255
            if dc == 1:
                win -= col0
            elif dc == -1:
                win -= col255
            if dr != 0 and dc != 0:
                win += pcorn[:, corner_map[(dr, dc)]]
            S_pred += dw[ki, kj] * win

    # ---- epilogue (tiny, float64) ----
    asm = S_pred[:, None, None] * S_feat[:, None, :]    # [B, 1, C]

    def ln(x, g, b):
        m = x.mean(-1, keepdims=True)
        v = ((x - m) ** 2).mean(-1, keepdims=True)
        return (x - m) / np.sqrt(v + LN_EPS) * g + b

    bs, ncls = head.shape[:2]
    head_r = np.asarray(head, np.float64).reshape(bs, ncls, C, 1).transpose(0, 1, 3, 2)
    bs_num = bs * ncls
    af = asm.reshape(-1, C)
    pf = af @ p['pt_w'].T + p['pt_b']
    pf_in, pf_out = pf[:, :FC], pf[:, FC:]
    hf = head_r.reshape(bs_num, -1, C) @ p['ht_w'].T + p['ht_b']
    hf_in, hf_out = hf[..., :FC], hf[..., FC:]
    gate = hf_in * pf_in[:, None, :]
    hg = _sigmoid(ln(gate @ p['hg_w'].T + p['hg_b'], p['ln_hin_g'], p['ln_hin_b']))
    pg = _sigmoid(ln(gate @ p['pg_w'].T + p['pg_b'], p['ln_pin_g'], p['ln_pin_b']))
    hf_out = ln(hf_out, p['ln_hout_g'], p['ln_hout_b'])
    pf_out = ln(pf_out, p['ln_pout_g'], p['ln_pout_b'])
    upd = pg * pf_out[:, None, :] + hg * hf_out
    upd = upd @ p['fc_w'].T + p['fc_b']
    upd = np.maximum(ln(upd, p['ln_fc_g'], p['ln_fc_b']), 0.0)
    upd = upd.reshape(bs, ncls, -1, FC).transpose(0, 1, 3, 2)
    return upd.reshape(bs, ncls, FC, 1, 1).astype(np.float32)


# revision 56
# speedup vs baseline: 1.1593x; 1.1593x over previous
"""Trainium2 Bass kernel for nn_EGMFA_12249246728581.

Mathematical reduction used here:
  The reference computes asm = irfft2(rfft2(pred_proc) * rfft2(feat_proc))
  summed over all spatial positions. The spatial sum of a circular
  convolution equals the product of the operands' spatial sums (DC Fourier
  coefficient), so asm[b,n,c] = S_pred[b,n] * S_feat[b,c].

  The spatial sum of a zero-padded depthwise conv collapses via
  inclusion-exclusion to a weighted combination of the input's total sum,
  edge-row/col strip sums and corner pixels.  So the only O(H*W) work is:
    - per-channel row sums + edge-column strip sums of `feat`  (device)
    - bilinear x2 upsample + sigmoid chain of `pred`, reduced the same way
      (device)
  Everything else is O(C)-sized and is combined on host in float64.

Device program (SPMD, one batch element per core; ~47us of HBM streaming
is the roofline and everything else overlaps it):
  feat[b] viewed as [128, 32768] (partition p = channel p//2, row-half p%2)
    - streamed in 10 tail-shrinking chunks on the SP hwdge ring;
      per chunk one 2x-mode tensor_scalar accum gives the chunk total per
      partition, and strided reduces accumulate the first/last-5-column
      strips (needed for the horizontal dwconv window sums).
  pred[b] [128,128] (emitted first so it hides under the feat stream):
    - vertical bilinear x2 via two PE matmuls with interpolation matrices,
      horizontal via shifted scalar_tensor_tensor ops, sigmoid chain on
      ACT, all reductions fused into accum_out outputs.
  Edge-row sums, corner pixels and the last 6 local rows' strip
  contributions come from tiny host-side slices of the raw input.
"""

import numpy as np

BS, C, H, W = 8, 64, 256, 256
GC, FC = 8, 64
CID = C - 3 * GC  # 40
LN_EPS = 1e-5
N_CORES = 8
FREE = (C * H * W) // 128       # 32768 floats per partition

# local rows (of 256 floats) per chunk; every chunk reduces to one
# 2x-mode total per partition.  Edge-row sums (channel rows 0-4/251-255)
# and edge-column strips (cols 0-4/251-255, ~4% of the data, a gather
# pattern the DVE handles ~10x slower than the cost model claims) are
# computed on host from the raw input.  Tail-shrinking chunk sizes keep
# the post-stream device work to one tiny tensor_scalar + one short DMA.
CHUNK_ROWS = [18, 18, 18, 18, 18, 18, 8, 6, 4, 2]
N_CHUNKS = len(CHUNK_ROWS)
# stats columns: [0:9] chunk 0-8 totals, [9] chunk-9 total
STAT_COLS = N_CHUNKS            # 10

_nc_cache = None


def _upsample_mats():
    """U_e/U_o: [128,128] f32, out_even = U_e.T @ x, out_odd = U_o.T @ x.
    Half-pixel-center bilinear x2 (align_corners=False)."""
    ue = np.zeros((128, 128), np.float32)
    uo = np.zeros((128, 128), np.float32)
    for i in range(128):
        if i == 0:
            ue[0, 0] = 1.0
        else:
            ue[i - 1, i] = 0.25
            ue[i, i] = 0.75
        if i == 127:
            uo[127, 127] = 1.0
        else:
            uo[i, i] = 0.75
            uo[i + 1, i] = 0.25
    return ue, uo


def _build_program(reps=1):
    import concourse.tile as tile
    from concourse import bacc, mybir

    f32 = mybir.dt.float32
    AX = mybir.AxisListType.X
    ALU = mybir.AluOpType
    ACT_SIG = mybir.ActivationFunctionType.Sigmoid

    nc = bacc.Bacc(None, target_bir_lowering=False)
    feat_in = nc.dram_tensor("feat", [128, FREE], f32, kind="ExternalInput")
    # paux = [pred | U_even | U_odd] concatenated so one DMA loads all three
    paux_in = nc.dram_tensor("paux", [128, 384], f32, kind="ExternalInput")
    # fstat = [rsum cols | col strips] in one tile -> one tail DMA
    fstat_out = nc.dram_tensor("fstat", [128, STAT_COLS], f32, kind="ExternalOutput")
    psmall_out = nc.dram_tensor("psmall", [128, 16], f32, kind="ExternalOutput")

    with tile.TileContext(nc) as tc:
        with (
            tc.tile_pool(name="feat", bufs=3) as feat_pool,
            tc.tile_pool(name="feat_sm", bufs=5) as feat_sm_pool,
            tc.tile_pool(name="acc", bufs=1) as acc_pool,
            tc.tile_pool(name="tmp", bufs=2) as tmp_pool,
            tc.tile_pool(name="pred", bufs=1) as pred_pool,
            tc.tile_pool(name="psum", bufs=1, space="PSUM") as psum_pool,
        ):
          assert sum(CHUNK_ROWS) == 128

          for _rep in range(reps):
            # ---------------- pred pipeline (emitted first so all its
            # compute lands in engine idle time under the feat stream;
            # its DVE ops retire before chunk 0's DMA even completes) ----
            psmall = acc_pool.tile([128, 16], f32)
            nc.vector.memset(psmall[:], 0.0)

            # tiny load on the ACT hwdge ring, so the SP ring starts the
            # feat stream at t=0 with nothing in front of it.
            paux_t = pred_pool.tile([128, 384], f32)
            nc.scalar.dma_start(paux_t[:], paux_in[:, :])
            pred_t = paux_t[:, 0:128]
            ue_t = paux_t[:, 128:256]
            uo_t = paux_t[:, 256:384]

            a_e_p = psum_pool.tile([128, 128], f32)
            a_o_p = psum_pool.tile([128, 128], f32)
            nc.tensor.matmul(a_e_p[:], ue_t, pred_t, start=True, stop=True)
            nc.tensor.matmul(a_o_p[:], uo_t, pred_t, start=True, stop=True)
            a_e = pred_pool.tile([128, 128], f32)
            a_o = pred_pool.tile([128, 128], f32)
            nc.vector.tensor_copy(a_e[:], a_e_p[:])
            nc.vector.tensor_copy(a_o[:], a_o_p[:])

            # phases: (row parity, col parity); P = pred_up * 4/3, then
            # sigmoid(P * 0.75) = pred_up sigmoid.
            phase_src = {"ee": (a_e, "e"), "eo": (a_e, "o"),
                         "oe": (a_o, "e"), "oo": (a_o, "o")}
            phase_idx = {"ee": 0, "eo": 1, "oe": 2, "oo": 3}
            pa_tiles = {}
            for ph, (a, cpar) in phase_src.items():
                p_t = pred_pool.tile([128, 128], f32, tag=f"p_{ph}")
                if cpar == "e":
                    # out col j: j==0 -> A[:,0]*(4/3); j>0 -> A[:,j-1]/3 + A[:,j]
                    nc.vector.scalar_tensor_tensor(
                        p_t[:, 1:128], a[:, 0:127], 1.0 / 3.0, a[:, 1:128],
                        op0=ALU.mult, op1=ALU.add)
                    nc.vector.scalar_tensor_tensor(
                        p_t[:, 0:1], a[:, 0:1], 1.0 / 3.0, a[:, 0:1],
                        op0=ALU.mult, op1=ALU.add)
                else:
                    nc.vector.scalar_tensor_tensor(
                        p_t[:, 0:127], a[:, 1:128], 1.0 / 3.0, a[:, 0:127],
                        op0=ALU.mult, op1=ALU.add)
                    nc.vector.scalar_tensor_tensor(
                        p_t[:, 127:128], a[:, 127:128], 1.0 / 3.0, a[:, 127:128],
                        op0=ALU.mult, op1=ALU.add)
                i = phase_idx[ph]
                s_t = pred_pool.tile([128, 128], f32, tag=f"s_{ph}")
                # pred_s = sigmoid(0.75*P); accum -> total per partition
                nc.scalar.activation(s_t[:], p_t[:], ACT_SIG, scale=0.75,
                                     accum_out=psmall[:, 4 + i:5 + i])
                g_t = pred_pool.tile([128, 128], f32, tag=f"g_{ph}")
                nc.scalar.activation(g_t[:], s_t[:], ACT_SIG)  # sigmoid(pred_s)
                t2 = pred_pool.tile([128, 128], f32, tag=f"t2_{ph}")
                nc.vector.tensor_mul(t2[:], s_t[:], g_t[:])    # pred_s * sig2
                pa = pred_pool.tile([128, 128], f32, tag=f"pa_{ph}")
                # pa = 2*pred_s - pred_s*sig2; accum -> pa row sums
                nc.vector.scalar_tensor_tensor(
                    pa[:], s_t[:], 2.0, t2[:],
                    op0=ALU.mult, op1=ALU.subtract,
                    accum_out=psmall[:, i:i + 1])
                pa_tiles[ph] = pa

            # edge columns of pa (col 0 from even-col phases, col 255 from odd)
            nc.vector.tensor_copy(psmall[:, 8:9], pa_tiles["ee"][:, 0:1])
            nc.vector.tensor_copy(psmall[:, 9:10], pa_tiles["oe"][:, 0:1])
            nc.vector.tensor_copy(psmall[:, 10:11], pa_tiles["eo"][:, 127:128])
            nc.vector.tensor_copy(psmall[:, 11:12], pa_tiles["oo"][:, 127:128])
            # psmall is complete well before the feat stream ends;
            # Pool (SWDGE) ring keeps it off the busy SP/ACT rings.
            nc.gpsimd.dma_start(psmall_out[:, :], psmall[:])
            del pa_tiles

            # ---------------- feat reduction ----------------
            stats = acc_pool.tile([128, STAT_COLS], f32)

            row0 = 0
            for k, nrows in enumerate(CHUNK_ROWS):
                fw = nrows * 256
                if nrows >= 16:
                    ch = feat_pool.tile([128, fw], f32, tag="ch_big")
                else:
                    ch = feat_sm_pool.tile([128, fw], f32, tag="ch_small")
                nc.sync.dma_start(ch[:], feat_in[:, row0 * 256:(row0 + nrows) * 256])
                # one scalar per partition, 2x-mode tensor_scalar
                scratch = tmp_pool.tile([128, 18 * 256], f32, tag="scratch")
                nc.vector.tensor_scalar(
                    scratch[:, 0:fw], ch[:], 1.0, 0.0,
                    op0=ALU.mult, op1=ALU.add,
                    accum_out=stats[:, k:k + 1])
                # stream completed totals out on the ACT hwdge ring
                if k < 6:
                    nc.scalar.dma_start(fstat_out[:, k:k + 1],
                                        stats[:, k:k + 1])
                row0 += nrows

            # bulk tail (ready once chunk 8 retires), then the short last
            # column that only depends on the final 2-row chunk's total.
            nc.scalar.dma_start(fstat_out[:, 6:STAT_COLS - 1],
                                stats[:, 6:STAT_COLS - 1])
            nc.scalar.dma_start(fstat_out[:, STAT_COLS - 1:STAT_COLS],
                                stats[:, STAT_COLS - 1:STAT_COLS])

    nc.compile()
    return nc


def _get_program():
    global _nc_cache
    if _nc_cache is None:
        _nc_cache = _build_program()
    return _nc_cache


def _sigmoid(x):
    return 1.0 / (1.0 + np.exp(-x))





def kernel(feat, head, pred, params):
    from concourse.bass_utils import run_bass_kernel_spmd

    feat = np.ascontiguousarray(feat, np.float32)
    head = np.asarray(head)
    pred = np.ascontiguousarray(pred, np.float32)

    ue, uo = _upsample_mats()
    in_maps = []
    for b in range(N_CORES):
        in_maps.append({
            "feat": feat[b].reshape(128, FREE),
            "paux": np.concatenate([pred[b, 0], ue, uo], axis=1),
        })

    nc = _get_program()
    results = run_bass_kernel_spmd(nc, in_maps, list(range(N_CORES))).results

    fstat = np.stack([r["fstat"] for r in results]).astype(np.float64)
    psmall = np.stack([r["psmall"] for r in results]).astype(np.float64)

    p = {k: np.asarray(v, np.float64) for k, v in params.items()}

    # ---- reassemble feat sums ----
    # fstat = per-chunk totals; partition 2c = channel c rows 0-127,
    # partition 2c+1 = rows 128-255.
    Th = fstat.sum(-1)                             # [B, 128] per half-channel
    T = Th[:, 0::2] + Th[:, 1::2]                  # [B, C]
    # edge-row sums and edge-column strips from the raw input (~4% of it;
    # a gather pattern the host handles better than the device engines)
    rows_lo = feat[:, :, 0:5, :].astype(np.float64).sum(-1)    # rows 0..4
    rows_hi = feat[:, :, 251:256, :].astype(np.float64).sum(-1)  # 251..255
    cs = np.empty((BS, C, 10))
    cs[..., 0:5] = feat[:, :, :, 0:5].astype(np.float64).sum(2)
    cs[..., 5:10] = feat[:, :, :, 251:256].astype(np.float64).sum(2)

    S_feat = T.copy()

    # y_hw: channels 40-47, 3x3 pad 1
    for ci in range(GC):
        c = CID + ci
        w = p['inc_hw_w'][ci, 0]
        s = np.zeros(BS)
        for ki in range(3):
            for kj in range(3):
                dr, dc = ki - 1, kj - 1
                win = T[:, c].copy()
                if dr == 1:
                    win -= rows_lo[:, c, 0]
                elif dr == -1:
                    win -= rows_hi[:, c, 4]
                if dc == 1:
                    win -= cs[:, c, 0]
                elif dc == -1:
                    win -= cs[:, c, 9]
                if dr != 0 and dc != 0:
                    rr = 0 if dr == 1 else 255
                    cc = 0 if dc == 1 else 255
                    win += feat[:, c, rr, cc].astype(np.float64)
                s += w[ki, kj] * win
        S_feat[:, c] = s + H * W * p['inc_hw_b'][ci]

    # y_w: channels 48-55, 1x11 pad (0,5)
    for ci in range(GC):
        c = CID + GC + ci
        w = p['inc_w_w'][ci, 0, 0]                 # [11]
        s = np.zeros(BS)
        for kj in range(11):
            d = kj - 5
            win = T[:, c].copy()
            if d > 0:
                win -= cs[:, c, :d].sum(-1)        # cols 0..d-1
            elif d < 0:
                win -= cs[:, c, 10 + d:].sum(-1)   # cols 256+d..255
            s += w[kj] * win
        S_feat[:, c] = s + H * W * p['inc_w_b'][ci]

    # y_h: channels 56-63, 11x1 pad (5,0)
    for ci in range(GC):
        c = CID + 2 * GC + ci
        w = p['inc_h_w'][ci, 0, :, 0]              # [11]
        s = np.zeros(BS)
        for ki in range(11):
            d = ki - 5
            win = T[:, c].copy()
            if d > 0:
                win -= rows_lo[:, c, :d].sum(-1)
            elif d < 0:
                win -= rows_hi[:, c, 5 + d:].sum(-1)
            s += w[ki] * win
        S_feat[:, c] = s + H * W * p['inc_h_b'][ci]

    # ---- pred scalar ----
    T_ps = psmall[:, :, 4:8].sum((1, 2))           # total of sigmoid(up)
    T_pa = psmall[:, :, 0:4].sum((1, 2))
    row0 = psmall[:, 0, 0] + psmall[:, 0, 1]
    row255 = psmall[:, 127, 2] + psmall[:, 127, 3]
    col0 = psmall[:, :, 8:10].sum((1, 2))
    col255 = psmall[:, :, 10:12].sum((1, 2))
    pv = pred[:, 0][:, [0, 0, 127, 127], [0, 127, 0, 127]].astype(np.float64)
    sc = _sigmoid(pv)
    pcorn = sc * (1.0 - _sigmoid(sc)) + sc         # pa at the 4 corners

    dw = p['dw_w'][0, 0]                           # [3,3]
    S_pred = T_ps + H * W * p['dw_b'][0]
    corner_map = {(1, 1): 0, (1, -1): 1, (-1, 1): 2, (-1, -1): 3}
    for ki in range(3):
        for kj in range(3):
            dr, dc = ki - 1, kj - 1
            win = T_pa.copy()
            if dr == 1:
                win -= row0
            elif dr == -1:
                win -= row255
            if dc == 1:
                win -= col0
            elif dc == -1:
                win -= col255
            if dr != 0 and dc != 0:
                win += pcorn[:, corner_map[(dr, dc)]]
            S_pred += dw[ki, kj] * win

    # ---- epilogue (tiny, float64) ----
    asm = S_pred[:, None, None] * S_feat[:, None, :]    # [B, 1, C]

    def ln(x, g, b):
        m = x.mean(-1, keepdims=True)
        v = ((x - m) ** 2).mean(-1, keepdims=True)
        return (x - m) / np.sqrt(v + LN_EPS) * g + b

    bs, ncls = head.shape[:2]
    head_r = np.asarray(head, np.float64).reshape(bs, ncls, C, 1).transpose(0, 1, 3, 2)
    bs_num = bs * ncls
    af = asm.reshape(-1, C)
    pf = af @ p['pt_w'].T + p['pt_b']
    pf_in, pf_out = pf[:, :FC], pf[:, FC:]
    hf = head_r.reshape(bs_num, -1, C) @ p['ht_w'].T + p['ht_b']
    hf_in, hf_out = hf[..., :FC], hf[..., FC:]
    gate = hf_in * pf_in[:, None, :]
    hg = _sigmoid(ln(gate @ p['hg_w'].T + p['hg_b'], p['ln_hin_g'], p['ln_hin_b']))
    pg = _sigmoid(ln(gate @ p['pg_w'].T + p['pg_b'], p['ln_pin_g'], p['ln_pin_b']))
    hf_out = ln(hf_out, p['ln_hout_g'], p['ln_hout_b'])
    pf_out = ln(pf_out, p['ln_pout_g'], p['ln_pout_b'])
    upd = pg * pf_out[:, None, :] + hg * hf_out
    upd = upd @ p['fc_w'].T + p['fc_b']
    upd = np.maximum(ln(upd, p['ln_fc_g'], p['ln_fc_b']), 0.0)
    upd = upd.reshape(bs, ncls, -1, FC).transpose(0, 1, 3, 2)
    return upd.reshape(bs, ncls, FC, 1, 1).astype(np.float32)
